# revision 35
# baseline (speedup 1.0000x reference)
"""Trainium2 Bass kernel for nn_CutoffModule (CBAM-style channel gate + topk gather).

Reference computation (per sample):
    avg/max spatial pooling -> shared 2-layer MLP -> sum -> sigmoid -> attn [C, D]
    per scale d: top-128 channels (sorted desc) -> gather those channels of x.

Sharding: data-parallel over N across 8 cores (4 samples/core); MLP weights
replicated. Entirely self-contained: hardcodes N=32, C=512, H=W=64, D=4, r=16.

Strategy: x is read from HBM once and kept in SBUF; the kernel computes the
INVERSE permutation (channel -> output rank, OOB sentinel when unselected) and
scatters x tiles straight to per-(sample, scale) output tensors with
indirect_dma_start (out_offset + bounds_check skip).  64 MiB HBM traffic/core
instead of the gather baseline's 96 MiB.

Samples are processed in asymmetric groups {0,1,2} then {3}: the big group's
24 MiB scatter keeps HBM busy through the small group's topk window.  SBUF
holds 10 retained x tiles + 1 streaming tile; sample 2's last two tiles are
pooled from the stream and re-loaded later for their scatter.

Notes:
- sigmoid is strictly monotonic, so top_k(sigmoid(y)) == top_k(y).
- topk row (d, li) lives on SBUF partition 32*d + li (engine writes must
  start at partition multiples of 32).
- inverse permutation per (sample, scale): one-hot is_equal + rank/selected
  matmul against [iota128, ones]; unselected channels get row id BIG=200 and
  are skipped by the DMA bounds check.
- every (sample, scale) block is a separate DRAM tensor: scatters to a shared
  tensor are WAW-chained by the tile framework and serialize.
- gpsimd (Pool) issues all scatters; it is a slow DSP for compute and cannot
  touch PSUM, so all element-wise work stays on DVE/ACT.
"""

import numpy as np

import concourse.bacc as bacc
import concourse.bass as bass
import concourse.tile as tile
from concourse import mybir
from concourse.bass_utils import run_bass_kernel_spmd

# Problem constants (hardcoded per harness contract)
N_FULL = 32
C = 512
HW = 64 * 64          # 4096
D = 4                 # depth scales
BLOCK = C // D        # 128
HID = C // 16         # 32  (MLP hidden)
N_CORES = 8
NS = N_FULL // N_CORES  # 4 samples per core
P = 128               # SBUF partitions
CT = C // P           # 4 channel tiles per sample
NEG_FILL = -1e30
BIG = 200.0           # OOB sentinel offset (> BLOCK-1)
XBUFS = 10            # retained x tile buffers (16 KiB/partition each)
AVGCH = 4             # avg-pool sub-chunks per tile (shrinks the ACT scratch)
NSWQ = 2              # SWDGE queues for scatters (round-robin)

GROUPS = [(0, 1, 2), (3,)]

F32 = mybir.dt.float32
U32 = mybir.dt.uint32


def _indirect_scatter_q(eng, out, offset_ap, in_, bounds, queue_num):
    """indirect_dma_start (out_offset form) with a selectable SWDGE queue.

    Replica of bass.BassEngine.indirect_dma_start's scatter path; the library
    hardcodes queue 0 ("qPoolDynamic"), which serializes all scatters behind
    one 8-deep completion-semaphore ring.
    """
    mb = mybir
    assert isinstance(out.offset, int) and out.offset == 0
    out_ap = eng.lower_ap_dma(out, for_indirect_dma=True)
    in_ap = eng.lower_ap_dma(in_, for_indirect_dma=True)
    assert len(in_ap) == 1 and len(out_ap) == 1
    off = eng.lower_ap_dma(offset_ap)
    assert len(off) == 1
    in_ap.append(off[0])

    coef = 1
    for s in out.shape[1:]:
        coef *= s
    out_ap[0].dynamic_ap_info = mb.DynamicAccessPatternInfo(
        c=0,
        actual_ap=in_.ap,
        indirect_dim_max_index=out.shape[0],
        offset_expr=[
            mb.DynamicAccessPatternOffsetExpr(
                coef=coef,
                aff_expr=mb.DynamicAccessPatternOffsetExprAffExpr(
                    kind="IndirectArgId", arg_id=1
                ),
            )
        ],
    )
    bc = [eng.lower_val_access(eng.to_reg(bounds))]
    qname = f"qPoolDynamic{queue_num or ''}"
    return eng.add_instruction(
        mb.InstDMACopy(
            name=eng.bass.get_next_instruction_name(),
            queue=qname,
            mode="Copy",
            ins=in_ap + bc,
            outs=out_ap,
            oob_is_err=False,
            cce_op=mb.AluOpType.bypass,
        )
    )


def _build_program():
    nc = bacc.Bacc(
        "TRN2", target_bir_lowering=False, debug=False, num_swdge_queues=NSWQ
    )

    x_d = nc.dram_tensor("x", [NS * C, HW], F32, kind="ExternalInput").ap()
    w1_d = nc.dram_tensor("w1", [C, HID], F32, kind="ExternalInput").ap()
    b1_d = nc.dram_tensor("b1", [HID, 1], F32, kind="ExternalInput").ap()
    # w2aug = [W2; 2*b2] so layer 2 + both bias adds fold into one K=33 matmul
    w2_d = nc.dram_tensor("w2aug", [HID + 1, C * D], F32, kind="ExternalInput").ap()
    ident_d = nc.dram_tensor("ident", [P, P], F32, kind="ExternalInput").ap()
    iota_d = nc.dram_tensor("iota512", [P, C], F32, kind="ExternalInput").ap()
    rhs2_d = nc.dram_tensor("rhs2", [P, 2], F32, kind="ExternalInput").ap()
    out_d = [
        [
            nc.dram_tensor(f"out_{n}_{d}", [BLOCK, HW], F32, kind="ExternalOutput").ap()
            for d in range(D)
        ]
        for n in range(NS)
    ]

    with tile.TileContext(nc) as tc:
        with (
            tc.tile_pool(name="xin", bufs=XBUFS) as xpool,
            tc.tile_pool(name="xstream", bufs=1) as spool,
            tc.tile_pool(name="small", bufs=1) as sm,
            tc.tile_pool(name="mbuf", bufs=2) as mpool,
            tc.tile_pool(name="psum", bufs=1, space="PSUM") as psum,
            tc.tile_pool(name="psumr", bufs=2, space="PSUM") as psumr,
        ):
            # ---- constants / weights into SBUF on the scalar(ACT) ring ----
            w1_sb = sm.tile([P, CT, HID], F32)
            nc.scalar.dma_start(
                out=w1_sb[:], in_=w1_d.rearrange("(c p) m -> p c m", p=P)
            )
            w2_sb = sm.tile([HID + 1, C * D], F32)
            nc.scalar.dma_start(out=w2_sb[:], in_=w2_d)
            b1_sb = sm.tile([HID, 1], F32)
            nc.scalar.dma_start(out=b1_sb[:], in_=b1_d)
            ident_sb = sm.tile([P, P], F32)
            nc.scalar.dma_start(out=ident_sb[:], in_=ident_d)
            iota_sb = sm.tile([P, C], F32)
            nc.scalar.dma_start(out=iota_sb[:], in_=iota_d)
            rhs2_sb = sm.tile([P, 2], F32)
            nc.scalar.dma_start(out=rhs2_sb[:], in_=rhs2_d)

            # [P, ct, {avg partials AVGCH*li+c, max cols nS*AVGCH+li}]
            nsz = [len(g) for g in GROUPS]
            pools = [
                sm.tile([P, CT, nsz[g] * (AVGCH + 1)], F32, name=f"pools{g}")
                for g in range(2)
            ]
            scratch = sm.tile([P, HW // AVGCH], F32)

            # one vals double-buffer shared by both groups (group 1 rewrites
            # the live rows; leftover NEG_FILLs elsewhere are unused)
            vals = [sm.tile([P, C], F32, name=f"vals_{i}") for i in range(2)]
            nc.vector.memset(vals[0][:], 0.0)

            offs_u = [
                sm.tile([P, nsz[g] * D, CT], U32, name=f"offs{g}") for g in range(2)
            ]

            xtiles = {}

            def load_tile(n, ct, eng, stream=False):
                row0 = (n * CT + ct) * P
                pool = spool if stream else xpool
                xt = pool.tile([P, HW], F32, tag="xs" if stream else "xt")
                xtiles[(n, ct)] = xt
                eng.dma_start(out=xt[:], in_=x_d[row0 : row0 + P, :])

            def avg_tile(n, ct, g, li):
                csz = HW // AVGCH
                xt = xtiles[(n, ct)]
                for c in range(AVGCH):
                    col = AVGCH * li + c
                    nc.scalar.activation(
                        out=scratch[:],
                        in_=xt[:, c * csz : (c + 1) * csz],
                        func=mybir.ActivationFunctionType.Copy,
                        scale=1.0 / HW,
                        accum_out=pools[g][:, ct, col : col + 1],
                    )

            def max_tile(n, ct, g, li):
                mc = nsz[g] * AVGCH + li
                nc.vector.reduce_max(
                    out=pools[g][:, ct, mc : mc + 1],
                    in_=xtiles[(n, ct)][:],
                    axis=mybir.AxisListType.X,
                )

            def mlp_group(g):
                """py rows 32d+li = y[sample li of group g]."""
                nS = nsz[g]
                ncols = nS * (AVGCH + 1)
                ph = psum.tile([HID, ncols], F32, space="PSUM", tag="ph")
                for ct in range(CT):
                    nc.tensor.matmul(
                        out=ph[:],
                        lhsT=w1_sb[:, ct, :],
                        rhs=pools[g][:, ct, :],
                        start=(ct == 0),
                        stop=(ct == CT - 1),
                    )
                # W1.T is linear: sum the avg partial columns after the matmul
                avgs = sm.tile([HID, nS], F32, name=f"avgs{g}")
                for li in range(nS):
                    nc.vector.reduce_sum(
                        out=avgs[:, li : li + 1],
                        in_=ph[:, AVGCH * li : AVGCH * (li + 1)],
                        axis=mybir.AxisListType.X,
                    )
                hTa = sm.tile([HID, nS], F32, name=f"hTa{g}")
                hTm = sm.tile([HID, nS], F32, name=f"hTm{g}")
                nc.scalar.activation(
                    out=hTa[:], in_=avgs[:],
                    func=mybir.ActivationFunctionType.Relu, bias=b1_sb[:, :],
                )
                nc.scalar.activation(
                    out=hTm[:], in_=ph[:, nS * AVGCH : nS * AVGCH + nS],
                    func=mybir.ActivationFunctionType.Relu, bias=b1_sb[:, :],
                )
                hsum = sm.tile([HID, nS], F32, name=f"hsum{g}")
                nc.vector.tensor_add(out=hsum[:], in0=hTa[:], in1=hTm[:])
                hw_t = sm.tile([HID + 1, P], F32, name=f"hw{g}")
                nc.vector.memset(hw_t[:], 0.0)
                nc.vector.memset(hw_t[32:33, :], 1.0)
                for d in range(D):
                    nc.vector.tensor_copy(
                        out=hw_t[0:HID, 32 * d : 32 * d + nS], in_=hsum[:]
                    )

                py = psum.tile([P, C * D], F32, space="PSUM", tag="py")
                for s in range(C * D // 512):
                    sl = slice(s * 512, (s + 1) * 512)
                    nc.tensor.matmul(
                        out=py[:, sl], lhsT=hw_t[:], rhs=w2_sb[:, sl],
                        start=True, stop=True,
                    )
                va = vals[0]
                for d in range(D):
                    nc.vector.tensor_copy(
                        out=va[32 * d : 32 * d + nS, :],
                        in_=py[32 * d : 32 * d + nS, d :: D],
                    )

            def topk_group(g):
                """ptf column 32d+li = topk channel ids (rank k on partitions)."""
                topk_idx = sm.tile([P, BLOCK], U32, name=f"tki{g}")
                maxv = sm.tile([P, 8], F32, name=f"maxv{g}")
                cur, nxt = vals
                for k in range(BLOCK // 8):
                    nc.vector.max(out=maxv[:], in_=cur[:])
                    nc.vector.max_index(
                        out=topk_idx[:, 8 * k : 8 * k + 8],
                        in_max=maxv[:],
                        in_values=cur[:],
                    )
                    if k < BLOCK // 8 - 1:
                        nc.vector.match_replace(
                            out=nxt[:], in_to_replace=maxv[:], in_values=cur[:],
                            imm_value=NEG_FILL,
                        )
                        cur, nxt = nxt, cur

                idx_f = sm.tile([P, BLOCK], F32, name=f"idxf{g}")
                nc.vector.tensor_copy(out=idx_f[:], in_=topk_idx[:])
                pt = psum.tile([P, P], F32, space="PSUM", tag="pt")
                nc.tensor.transpose(out=pt[:], in_=idx_f[:], identity=ident_sb[:])
                ptf = sm.tile([P, P], F32, name=f"ptf{g}")
                nc.vector.tensor_copy(out=ptf[:], in_=pt[:])
                return ptf

            def inverse_group(g, ptf):
                """offs_u[g][:, li*D+d, ct] = output row per channel."""
                for li in range(nsz[g]):
                    for d in range(D):
                        col = 32 * d + li
                        j = li * D + d
                        m = mpool.tile([P, C], F32, tag="m")
                        nc.vector.tensor_scalar(
                            out=m[:], in0=iota_sb[:],
                            scalar1=ptf[:, col : col + 1], scalar2=None,
                            op0=mybir.AluOpType.is_equal,
                        )
                        psR = psumr.tile([P, CT, 2], F32, space="PSUM", tag="psr")
                        for ct in range(CT):
                            nc.tensor.matmul(
                                out=psR[:, ct, :],
                                lhsT=m[:, ct * P : (ct + 1) * P],
                                rhs=rhs2_sb[:],
                                start=True, stop=True,
                            )
                        # offs = rank*sel + BIG*(1-sel)
                        tmp = sm.tile([P, CT], F32, name=f"tmp{g}")
                        nc.vector.tensor_scalar(
                            out=tmp[:], in0=psR[:, :, 1],
                            scalar1=-BIG, scalar2=BIG,
                            op0=mybir.AluOpType.mult,
                            op1=mybir.AluOpType.add,
                        )
                        offs_f = sm.tile([P, CT], F32, name=f"offsf{g}")
                        nc.vector.tensor_add(
                            out=offs_f[:], in0=tmp[:], in1=psR[:, :, 0]
                        )
                        nc.vector.tensor_copy(
                            out=offs_u[g][:, j, :], in_=offs_f[:]
                        )

            sc_counter = [0]

            def scatter_one(g, li, n, d, ct):
                j = li * D + d
                qn = sc_counter[0] % NSWQ
                sc_counter[0] += 1
                _indirect_scatter_q(
                    nc.gpsimd,
                    out=out_d[n][d][:, :],
                    offset_ap=offs_u[g][:, j, ct : ct + 1],
                    in_=xtiles[(n, ct)][:],
                    bounds=BLOCK - 1,
                    queue_num=qn,
                )

            # ================= emission (== engine program order) ===========
            # group A loads: s0, s1, s2ct0 retained (even ct sync, odd ACT);
            # s2ct1/ct2 stream through the 2 stream buffers, ct3 below (its
            # load must follow ct1's pooling to reuse that buffer)
            # stream chain (one buffer): s2ct1 -> s2ct2 -> s3ct0; each load
            # waits the previous tile's pooling.  Stream loads sit early on
            # their rings so that chain starts immediately.
            load_tile(2, 1, nc.scalar, stream=True)
            load_tile(0, 0, nc.sync)
            load_tile(0, 1, nc.scalar)
            load_tile(2, 2, nc.sync, stream=True)
            load_tile(0, 2, nc.sync)
            load_tile(0, 3, nc.scalar)
            for ct in range(CT):
                load_tile(1, ct, nc.scalar if ct % 2 else nc.sync)
            load_tile(2, 0, nc.sync)
            load_tile(2, 3, nc.scalar)
            # s3ct0 streams in during group A's topk; resident for scatter B
            load_tile(3, 0, nc.sync, stream=True)

            # streamed tiles' pooling first: it gates stream-buffer reuse
            avg_tile(2, 1, 0, 2)
            avg_tile(2, 2, 0, 2)
            for li, n in enumerate((0, 1)):
                for ct in range(CT):
                    avg_tile(n, ct, 0, li)
            avg_tile(2, 3, 0, 2)
            avg_tile(2, 0, 0, 2)
            max_tile(2, 1, 0, 2)
            max_tile(2, 2, 0, 2)
            for li, n in enumerate((0, 1)):
                for ct in range(CT):
                    max_tile(n, ct, 0, li)
            max_tile(2, 3, 0, 2)
            max_tile(2, 0, 0, 2)

            mlp_group(0)
            ptf0 = topk_group(0)
            inverse_group(0, ptf0)

            # gated loads: s3's remainder first (pooling is tail-critical),
            # then the s2ct1/ct2 reloads (only needed late in scatter A)
            load_tile(3, 1, nc.sync)
            load_tile(3, 2, nc.sync)
            load_tile(3, 3, nc.scalar)
            for ct in (1, 2):
                load_tile(2, ct, nc.sync)           # reload into retained pool
            for ct in range(CT):
                avg_tile(3, ct, 1, 0)

            # scatter A: s0/s1 ct-major round-robin, then s2 (resident ct0/ct3
            # first, reloaded ct1/ct2 last)
            for ct in range(CT):
                for li in (0, 1):
                    for d in range(D):
                        scatter_one(0, li, li, d, ct)
            for ct in (0, 3, 1, 2):
                for d in range(D):
                    scatter_one(0, 2, 2, d, ct)

            for ct in range(CT):
                max_tile(3, ct, 1, 0)
            mlp_group(1)
            ptf1 = topk_group(1)
            inverse_group(1, ptf1)
            for ct in range(CT):
                for d in range(D):
                    scatter_one(1, 0, 3, d, ct)

    nc.compile()
    return nc


_NC_CACHE = None


def _get_nc():
    global _NC_CACHE
    if _NC_CACHE is None:
        _NC_CACHE = _build_program()
    return _NC_CACHE


def _make_in_maps(x, W1, b1, W2, b2):
    x = np.ascontiguousarray(np.asarray(x, dtype=np.float32)).reshape(N_FULL, C, HW)
    W1 = np.asarray(W1, dtype=np.float32)
    b1 = np.asarray(b1, dtype=np.float32).reshape(HID, 1)
    W2 = np.asarray(W2, dtype=np.float32)
    b2 = np.asarray(b2, dtype=np.float32).reshape(1, C * D)
    w2aug = np.ascontiguousarray(np.vstack([W2, 2.0 * b2]))
    ident = np.eye(P, dtype=np.float32)
    iota512 = np.tile(np.arange(C, dtype=np.float32), (P, 1))
    rhs2 = np.stack(
        [np.arange(P, dtype=np.float32), np.ones(P, dtype=np.float32)], axis=1
    )
    rhs2 = np.ascontiguousarray(rhs2)
    in_maps = []
    for core in range(N_CORES):
        shard = x[core * NS : (core + 1) * NS].reshape(NS * C, HW)
        in_maps.append(
            {
                "x": np.ascontiguousarray(shard),
                "w1": W1,
                "b1": b1,
                "w2aug": w2aug,
                "ident": ident,
                "iota512": iota512,
                "rhs2": rhs2,
            }
        )
    return in_maps


def run(inputs, trace=False, **kwargs):
    """Run the SPMD kernel; returns (full_output, BassKernelResults)."""
    nc = _get_nc()
    in_maps = _make_in_maps(
        inputs["x"], inputs["W1"], inputs["b1"], inputs["W2"], inputs["b2"]
    )
    res = run_bass_kernel_spmd(
        nc, in_maps, core_ids=list(range(N_CORES)), trace=trace, **kwargs
    )
    out = np.empty((N_FULL, C, 64, 64), dtype=np.float32)
    for core in range(N_CORES):
        r = res.results[core]
        for n in range(NS):
            for d in range(D):
                blk = r[f"out_{n}_{d}"].reshape(BLOCK, 64, 64)
                out[core * NS + n, d * BLOCK : (d + 1) * BLOCK] = blk
    return out, res


def kernel(**inputs) -> np.ndarray:
    out, _ = run(inputs)
    return out


# revision 37
# speedup vs baseline: 1.0803x; 1.0803x over previous
"""Trainium2 Bass kernel for nn_CutoffModule (CBAM-style channel gate + topk gather).

Reference computation (per sample):
    avg/max spatial pooling -> shared 2-layer MLP -> sum -> sigmoid -> attn [C, D]
    per scale d: top-128 channels (sorted desc) -> gather those channels of x.

Sharding: data-parallel over N across 8 cores (4 samples/core); MLP weights
replicated. Entirely self-contained: hardcodes N=32, C=512, H=W=64, D=4, r=16.

Strategy: x is read from HBM once and kept in SBUF; the kernel computes the
INVERSE permutation (channel -> output rank, OOB sentinel when unselected) and
scatters x tiles straight to per-(sample, scale) output tensors with
indirect_dma_start (out_offset + bounds_check skip).  64 MiB HBM traffic/core
instead of the gather baseline's 96 MiB.

Samples are processed in asymmetric groups {0,1,2} then {3}: the big group's
24 MiB scatter keeps HBM busy through the small group's topk window.  SBUF
holds 10 retained x tiles + 1 streaming tile; sample 2's last two tiles are
pooled from the stream and re-loaded later for their scatter.

Notes:
- sigmoid is strictly monotonic, so top_k(sigmoid(y)) == top_k(y).
- topk row (d, li) lives on SBUF partition 32*d + li (engine writes must
  start at partition multiples of 32).
- inverse permutation per (sample, scale): one-hot is_equal + rank/selected
  matmul against [iota128, ones]; unselected channels get row id BIG=200 and
  are skipped by the DMA bounds check.
- every (sample, scale) block is a separate DRAM tensor: scatters to a shared
  tensor are WAW-chained by the tile framework and serialize.
- gpsimd (Pool) issues all scatters; it is a slow DSP for compute and cannot
  touch PSUM, so all element-wise work stays on DVE/ACT.
"""

import numpy as np

import concourse.bacc as bacc
import concourse.bass as bass
import concourse.tile as tile
from concourse import mybir
from concourse.bass_utils import run_bass_kernel_spmd

# Problem constants (hardcoded per harness contract)
N_FULL = 32
C = 512
HW = 64 * 64          # 4096
D = 4                 # depth scales
BLOCK = C // D        # 128
HID = C // 16         # 32  (MLP hidden)
N_CORES = 8
NS = N_FULL // N_CORES  # 4 samples per core
P = 128               # SBUF partitions
CT = C // P           # 4 channel tiles per sample
NEG_FILL = -1e30
BIG = 200.0           # OOB sentinel offset (> BLOCK-1)
XBUFS = 9             # retained x tile buffers (16 KiB/partition each)
AVGCH = 4             # avg-pool sub-chunks per tile (shrinks the ACT scratch)
NSWQ = 2              # SWDGE queues for scatters (round-robin)

GROUPS = [(0, 1, 2), (3,)]

F32 = mybir.dt.float32
U32 = mybir.dt.uint32


def _indirect_scatter_q(eng, out, offset_ap, in_, bounds, queue_num):
    """indirect_dma_start (out_offset form) with a selectable SWDGE queue.

    Replica of bass.BassEngine.indirect_dma_start's scatter path; the library
    hardcodes queue 0 ("qPoolDynamic"), which serializes all scatters behind
    one 8-deep completion-semaphore ring.
    """
    mb = mybir
    assert isinstance(out.offset, int) and out.offset == 0
    out_ap = eng.lower_ap_dma(out, for_indirect_dma=True)
    in_ap = eng.lower_ap_dma(in_, for_indirect_dma=True)
    assert len(in_ap) == 1 and len(out_ap) == 1
    off = eng.lower_ap_dma(offset_ap)
    assert len(off) == 1
    in_ap.append(off[0])

    coef = 1
    for s in out.shape[1:]:
        coef *= s
    out_ap[0].dynamic_ap_info = mb.DynamicAccessPatternInfo(
        c=0,
        actual_ap=in_.ap,
        indirect_dim_max_index=out.shape[0],
        offset_expr=[
            mb.DynamicAccessPatternOffsetExpr(
                coef=coef,
                aff_expr=mb.DynamicAccessPatternOffsetExprAffExpr(
                    kind="IndirectArgId", arg_id=1
                ),
            )
        ],
    )
    bc = [eng.lower_val_access(eng.to_reg(bounds))]
    qname = f"qPoolDynamic{queue_num or ''}"
    return eng.add_instruction(
        mb.InstDMACopy(
            name=eng.bass.get_next_instruction_name(),
            queue=qname,
            mode="Copy",
            ins=in_ap + bc,
            outs=out_ap,
            oob_is_err=False,
            cce_op=mb.AluOpType.bypass,
        )
    )


def _build_program():
    nc = bacc.Bacc(
        "TRN2", target_bir_lowering=False, debug=False, num_swdge_queues=NSWQ
    )

    x_d = nc.dram_tensor("x", [NS * C, HW], F32, kind="ExternalInput").ap()
    w1_d = nc.dram_tensor("w1", [C, HID], F32, kind="ExternalInput").ap()
    b1_d = nc.dram_tensor("b1", [HID, 1], F32, kind="ExternalInput").ap()
    # w2aug = [W2; 2*b2] so layer 2 + both bias adds fold into one K=33 matmul
    w2_d = nc.dram_tensor("w2aug", [HID + 1, C * D], F32, kind="ExternalInput").ap()
    ident_d = nc.dram_tensor("ident", [P, P], F32, kind="ExternalInput").ap()
    iota_d = nc.dram_tensor("iota512", [P, C], F32, kind="ExternalInput").ap()
    rhs2_d = nc.dram_tensor("rhs2", [P, 2], F32, kind="ExternalInput").ap()
    out_d = [
        [
            nc.dram_tensor(f"out_{n}_{d}", [BLOCK, HW], F32, kind="ExternalOutput").ap()
            for d in range(D)
        ]
        for n in range(NS)
    ]

    with tile.TileContext(nc) as tc:
        with (
            tc.tile_pool(name="xin", bufs=XBUFS) as xpool,
            tc.tile_pool(name="xstream", bufs=2) as spool,
            tc.tile_pool(name="small", bufs=1) as sm,
            tc.tile_pool(name="mbuf", bufs=2) as mpool,
            tc.tile_pool(name="psum", bufs=1, space="PSUM") as psum,
            tc.tile_pool(name="psumr", bufs=2, space="PSUM") as psumr,
        ):
            # ---- constants / weights into SBUF on the scalar(ACT) ring ----
            w1_sb = sm.tile([P, CT, HID], F32)
            nc.scalar.dma_start(
                out=w1_sb[:], in_=w1_d.rearrange("(c p) m -> p c m", p=P)
            )
            w2_sb = sm.tile([HID + 1, C * D], F32)
            nc.scalar.dma_start(out=w2_sb[:], in_=w2_d)
            b1_sb = sm.tile([HID, 1], F32)
            nc.scalar.dma_start(out=b1_sb[:], in_=b1_d)
            ident_sb = sm.tile([P, P], F32)
            nc.scalar.dma_start(out=ident_sb[:], in_=ident_d)
            iota_sb = sm.tile([P, C], F32)
            nc.scalar.dma_start(out=iota_sb[:], in_=iota_d)
            rhs2_sb = sm.tile([P, 2], F32)
            nc.scalar.dma_start(out=rhs2_sb[:], in_=rhs2_d)

            # [P, ct, {avg partials AVGCH*li+c, max cols nS*AVGCH+li}]
            nsz = [len(g) for g in GROUPS]
            pools = [
                sm.tile([P, CT, nsz[g] * (AVGCH + 1)], F32, name=f"pools{g}")
                for g in range(2)
            ]
            scratch = sm.tile([P, HW // AVGCH], F32)

            # one vals double-buffer shared by both groups (group 1 rewrites
            # the live rows; leftover NEG_FILLs elsewhere are unused)
            vals = [sm.tile([P, C], F32, name=f"vals_{i}") for i in range(2)]
            nc.vector.memset(vals[0][:], 0.0)

            offs_u = [
                sm.tile([P, nsz[g] * D, CT], U32, name=f"offs{g}") for g in range(2)
            ]

            xtiles = {}

            def load_tile(n, ct, eng, stream=False):
                row0 = (n * CT + ct) * P
                pool = spool if stream else xpool
                xt = pool.tile([P, HW], F32, tag="xs" if stream else "xt")
                xtiles[(n, ct)] = xt
                eng.dma_start(out=xt[:], in_=x_d[row0 : row0 + P, :])

            def avg_tile(n, ct, g, li):
                csz = HW // AVGCH
                xt = xtiles[(n, ct)]
                for c in range(AVGCH):
                    col = AVGCH * li + c
                    nc.scalar.activation(
                        out=scratch[:],
                        in_=xt[:, c * csz : (c + 1) * csz],
                        func=mybir.ActivationFunctionType.Copy,
                        scale=1.0 / HW,
                        accum_out=pools[g][:, ct, col : col + 1],
                    )

            def max_tile(n, ct, g, li):
                mc = nsz[g] * AVGCH + li
                nc.vector.reduce_max(
                    out=pools[g][:, ct, mc : mc + 1],
                    in_=xtiles[(n, ct)][:],
                    axis=mybir.AxisListType.X,
                )

            def mlp_group(g):
                """py rows 32d+li = y[sample li of group g]."""
                nS = nsz[g]
                ncols = nS * (AVGCH + 1)
                ph = psum.tile([HID, ncols], F32, space="PSUM", tag="ph")
                for ct in range(CT):
                    nc.tensor.matmul(
                        out=ph[:],
                        lhsT=w1_sb[:, ct, :],
                        rhs=pools[g][:, ct, :],
                        start=(ct == 0),
                        stop=(ct == CT - 1),
                    )
                # W1.T is linear: sum the avg partial columns after the matmul
                avgs = sm.tile([HID, nS], F32, name=f"avgs{g}")
                for li in range(nS):
                    nc.vector.reduce_sum(
                        out=avgs[:, li : li + 1],
                        in_=ph[:, AVGCH * li : AVGCH * (li + 1)],
                        axis=mybir.AxisListType.X,
                    )
                hTa = sm.tile([HID, nS], F32, name=f"hTa{g}")
                hTm = sm.tile([HID, nS], F32, name=f"hTm{g}")
                nc.scalar.activation(
                    out=hTa[:], in_=avgs[:],
                    func=mybir.ActivationFunctionType.Relu, bias=b1_sb[:, :],
                )
                nc.scalar.activation(
                    out=hTm[:], in_=ph[:, nS * AVGCH : nS * AVGCH + nS],
                    func=mybir.ActivationFunctionType.Relu, bias=b1_sb[:, :],
                )
                hsum = sm.tile([HID, nS], F32, name=f"hsum{g}")
                nc.vector.tensor_add(out=hsum[:], in0=hTa[:], in1=hTm[:])
                hw_t = sm.tile([HID + 1, P], F32, name=f"hw{g}")
                nc.vector.memset(hw_t[:], 0.0)
                nc.vector.memset(hw_t[32:33, :], 1.0)
                for d in range(D):
                    nc.vector.tensor_copy(
                        out=hw_t[0:HID, 32 * d : 32 * d + nS], in_=hsum[:]
                    )

                py = psum.tile([P, C * D], F32, space="PSUM", tag="py")
                for s in range(C * D // 512):
                    sl = slice(s * 512, (s + 1) * 512)
                    nc.tensor.matmul(
                        out=py[:, sl], lhsT=hw_t[:], rhs=w2_sb[:, sl],
                        start=True, stop=True,
                    )
                va = vals[0]
                for d in range(D):
                    nc.vector.tensor_copy(
                        out=va[32 * d : 32 * d + nS, :],
                        in_=py[32 * d : 32 * d + nS, d :: D],
                    )

            def topk_group(g):
                """ptf column 32d+li = topk channel ids (rank k on partitions)."""
                topk_idx = sm.tile([P, BLOCK], U32, name=f"tki{g}")
                maxv = sm.tile([P, 8], F32, name=f"maxv{g}")
                cur, nxt = vals
                for k in range(BLOCK // 8):
                    nc.vector.max(out=maxv[:], in_=cur[:])
                    nc.vector.max_index(
                        out=topk_idx[:, 8 * k : 8 * k + 8],
                        in_max=maxv[:],
                        in_values=cur[:],
                    )
                    if k < BLOCK // 8 - 1:
                        nc.vector.match_replace(
                            out=nxt[:], in_to_replace=maxv[:], in_values=cur[:],
                            imm_value=NEG_FILL,
                        )
                        cur, nxt = nxt, cur

                idx_f = sm.tile([P, BLOCK], F32, name=f"idxf{g}")
                nc.vector.tensor_copy(out=idx_f[:], in_=topk_idx[:])
                pt = psum.tile([P, P], F32, space="PSUM", tag="pt")
                nc.tensor.transpose(out=pt[:], in_=idx_f[:], identity=ident_sb[:])
                ptf = sm.tile([P, P], F32, name=f"ptf{g}")
                nc.vector.tensor_copy(out=ptf[:], in_=pt[:])
                return ptf

            def inverse_group(g, ptf):
                """offs_u[g][:, li*D+d, ct] = output row per channel."""
                for li in range(nsz[g]):
                    for d in range(D):
                        col = 32 * d + li
                        j = li * D + d
                        m = mpool.tile([P, C], F32, tag="m")
                        nc.vector.tensor_scalar(
                            out=m[:], in0=iota_sb[:],
                            scalar1=ptf[:, col : col + 1], scalar2=None,
                            op0=mybir.AluOpType.is_equal,
                        )
                        psR = psumr.tile([P, CT, 2], F32, space="PSUM", tag="psr")
                        for ct in range(CT):
                            nc.tensor.matmul(
                                out=psR[:, ct, :],
                                lhsT=m[:, ct * P : (ct + 1) * P],
                                rhs=rhs2_sb[:],
                                start=True, stop=True,
                            )
                        # offs = rank*sel + BIG*(1-sel)
                        tmp = sm.tile([P, CT], F32, name=f"tmp{g}")
                        nc.vector.tensor_scalar(
                            out=tmp[:], in0=psR[:, :, 1],
                            scalar1=-BIG, scalar2=BIG,
                            op0=mybir.AluOpType.mult,
                            op1=mybir.AluOpType.add,
                        )
                        offs_f = sm.tile([P, CT], F32, name=f"offsf{g}")
                        nc.vector.tensor_add(
                            out=offs_f[:], in0=tmp[:], in1=psR[:, :, 0]
                        )
                        nc.vector.tensor_copy(
                            out=offs_u[g][:, j, :], in_=offs_f[:]
                        )

            sc_counter = [0]

            def scatter_one(g, li, n, d, ct):
                j = li * D + d
                qn = sc_counter[0] % NSWQ
                sc_counter[0] += 1
                _indirect_scatter_q(
                    nc.gpsimd,
                    out=out_d[n][d][:, :],
                    offset_ap=offs_u[g][:, j, ct : ct + 1],
                    in_=xtiles[(n, ct)][:],
                    bounds=BLOCK - 1,
                    queue_num=qn,
                )

            # ================= emission (== engine program order) ===========
            # group A loads: s0, s1, s2ct0 retained (even ct sync, odd ACT);
            # stream chain: bufA s2ct1 -> s2ct3 -> s3ct1, bufB s2ct2 -> s3ct0.
            # Stream loads go first on their rings so the chain starts
            # immediately; avg(2,1) is interleaved into the ACT load sequence
            # (before ld(2,3), which waits on it) so the chain's second hop
            # unblocks at ~15us.  Rings carry 6 group-A loads each.
            load_tile(2, 1, nc.scalar, stream=True)
            load_tile(2, 2, nc.sync, stream=True)
            load_tile(0, 0, nc.sync)
            load_tile(0, 1, nc.scalar)
            avg_tile(2, 1, 0, 2)
            load_tile(2, 3, nc.scalar, stream=True)  # waits avg/max(2,1)
            load_tile(0, 2, nc.sync)
            load_tile(0, 3, nc.scalar)
            load_tile(1, 0, nc.sync)
            load_tile(1, 1, nc.scalar)
            load_tile(1, 2, nc.sync)
            load_tile(1, 3, nc.scalar)
            load_tile(2, 0, nc.sync)
            # s3ct0/ct1 stream in during group A's topk (buffers free once
            # s2ct2/ct3 are pooled); they stay resident for scatter B
            load_tile(3, 0, nc.sync, stream=True)
            load_tile(3, 1, nc.sync, stream=True)

            # pooling in rough arrival order; streamed tiles first (they gate
            # the stream-buffer reuse chain)
            avg_tile(2, 2, 0, 2)
            for ct in range(CT):
                avg_tile(0, ct, 0, 0)
            avg_tile(2, 3, 0, 2)
            for ct in range(CT):
                avg_tile(1, ct, 0, 1)
            avg_tile(2, 0, 0, 2)
            max_tile(2, 1, 0, 2)
            max_tile(2, 2, 0, 2)
            for ct in range(CT):
                max_tile(0, ct, 0, 0)
            max_tile(2, 3, 0, 2)
            for ct in range(CT):
                max_tile(1, ct, 0, 1)
            max_tile(2, 0, 0, 2)

            mlp_group(0)
            ptf0 = topk_group(0)
            inverse_group(0, ptf0)

            # gated loads: s3's remainder first (pooling is tail-critical),
            # then the s2ct1-3 reloads (only needed late in scatter A)
            load_tile(3, 2, nc.sync)
            load_tile(3, 3, nc.scalar)
            for ct in (1, 2, 3):
                load_tile(2, ct, nc.sync)           # reload into retained pool
            for ct in range(CT):
                avg_tile(3, ct, 1, 0)

            # scatter A: s0/s1 ct-major round-robin, then s2 (reloads last)
            for ct in range(CT):
                for li in (0, 1):
                    for d in range(D):
                        scatter_one(0, li, li, d, ct)
            for ct in range(CT):
                for d in range(D):
                    scatter_one(0, 2, 2, d, ct)

            for ct in range(CT):
                max_tile(3, ct, 1, 0)
            mlp_group(1)
            ptf1 = topk_group(1)
            inverse_group(1, ptf1)
            for ct in range(CT):
                for d in range(D):
                    scatter_one(1, 0, 3, d, ct)

    nc.compile()
    return nc


_NC_CACHE = None


def _get_nc():
    global _NC_CACHE
    if _NC_CACHE is None:
        _NC_CACHE = _build_program()
    return _NC_CACHE


def _make_in_maps(x, W1, b1, W2, b2):
    x = np.ascontiguousarray(np.asarray(x, dtype=np.float32)).reshape(N_FULL, C, HW)
    W1 = np.asarray(W1, dtype=np.float32)
    b1 = np.asarray(b1, dtype=np.float32).reshape(HID, 1)
    W2 = np.asarray(W2, dtype=np.float32)
    b2 = np.asarray(b2, dtype=np.float32).reshape(1, C * D)
    w2aug = np.ascontiguousarray(np.vstack([W2, 2.0 * b2]))
    ident = np.eye(P, dtype=np.float32)
    iota512 = np.tile(np.arange(C, dtype=np.float32), (P, 1))
    rhs2 = np.stack(
        [np.arange(P, dtype=np.float32), np.ones(P, dtype=np.float32)], axis=1
    )
    rhs2 = np.ascontiguousarray(rhs2)
    in_maps = []
    for core in range(N_CORES):
        shard = x[core * NS : (core + 1) * NS].reshape(NS * C, HW)
        in_maps.append(
            {
                "x": np.ascontiguousarray(shard),
                "w1": W1,
                "b1": b1,
                "w2aug": w2aug,
                "ident": ident,
                "iota512": iota512,
                "rhs2": rhs2,
            }
        )
    return in_maps


def run(inputs, trace=False, **kwargs):
    """Run the SPMD kernel; returns (full_output, BassKernelResults)."""
    nc = _get_nc()
    in_maps = _make_in_maps(
        inputs["x"], inputs["W1"], inputs["b1"], inputs["W2"], inputs["b2"]
    )
    res = run_bass_kernel_spmd(
        nc, in_maps, core_ids=list(range(N_CORES)), trace=trace, **kwargs
    )
    out = np.empty((N_FULL, C, 64, 64), dtype=np.float32)
    for core in range(N_CORES):
        r = res.results[core]
        for n in range(NS):
            for d in range(D):
                blk = r[f"out_{n}_{d}"].reshape(BLOCK, 64, 64)
                out[core * NS + n, d * BLOCK : (d + 1) * BLOCK] = blk
    return out, res


def kernel(**inputs) -> np.ndarray:
    out, _ = run(inputs)
    return out


# revision 38
# speedup vs baseline: 1.1414x; 1.0566x over previous
"""Trainium2 Bass kernel for nn_CutoffModule (CBAM-style channel gate + topk gather).

Reference computation (per sample):
    avg/max spatial pooling -> shared 2-layer MLP -> sum -> sigmoid -> attn [C, D]
    per scale d: top-128 channels (sorted desc) -> gather those channels of x.

Sharding: data-parallel over N across 8 cores (4 samples/core); MLP weights
replicated. Entirely self-contained: hardcodes N=32, C=512, H=W=64, D=4, r=16.

Strategy: x is read from HBM once and kept in SBUF; the kernel computes the
INVERSE permutation (channel -> output rank, OOB sentinel when unselected) and
scatters x tiles straight to per-(sample, scale) output tensors with
indirect_dma_start (out_offset + bounds_check skip).  64 MiB HBM traffic/core
instead of the gather baseline's 96 MiB.

Samples are processed in asymmetric groups {0,1,2} then {3}: the big group's
24 MiB scatter keeps HBM busy through the small group's topk window.  SBUF
holds 10 retained x tiles + 1 streaming tile; sample 2's last two tiles are
pooled from the stream and re-loaded later for their scatter.

Notes:
- sigmoid is strictly monotonic, so top_k(sigmoid(y)) == top_k(y).
- topk row (d, li) lives on SBUF partition 32*d + li (engine writes must
  start at partition multiples of 32).
- inverse permutation per (sample, scale): one-hot is_equal + rank/selected
  matmul against [iota128, ones]; unselected channels get row id BIG=200 and
  are skipped by the DMA bounds check.
- every (sample, scale) block is a separate DRAM tensor: scatters to a shared
  tensor are WAW-chained by the tile framework and serialize.
- gpsimd (Pool) issues all scatters; it is a slow DSP for compute and cannot
  touch PSUM, so all element-wise work stays on DVE/ACT.
"""

import numpy as np

import concourse.bacc as bacc
import concourse.bass as bass
import concourse.tile as tile
from concourse import mybir
from concourse.bass_utils import run_bass_kernel_spmd

# Problem constants (hardcoded per harness contract)
N_FULL = 32
C = 512
HW = 64 * 64          # 4096
D = 4                 # depth scales
BLOCK = C // D        # 128
HID = C // 16         # 32  (MLP hidden)
N_CORES = 8
NS = N_FULL // N_CORES  # 4 samples per core
P = 128               # SBUF partitions
CT = C // P           # 4 channel tiles per sample
NEG_FILL = -1e30
BIG = 200.0           # OOB sentinel offset (> BLOCK-1)
XBUFS = 9             # retained x tile buffers (16 KiB/partition each)
AVGCH = 4             # avg-pool sub-chunks per tile (shrinks the ACT scratch)
NSWQ = 2              # SWDGE queues for scatters (round-robin)

GROUPS = [(0, 1, 2), (3,)]

F32 = mybir.dt.float32
U32 = mybir.dt.uint32


def _indirect_scatter_q(eng, out, offset_ap, in_, bounds, queue_num):
    """indirect_dma_start (out_offset form) with a selectable SWDGE queue.

    Replica of bass.BassEngine.indirect_dma_start's scatter path; the library
    hardcodes queue 0 ("qPoolDynamic"), which serializes all scatters behind
    one 8-deep completion-semaphore ring.
    """
    mb = mybir
    assert isinstance(out.offset, int) and out.offset == 0
    out_ap = eng.lower_ap_dma(out, for_indirect_dma=True)
    in_ap = eng.lower_ap_dma(in_, for_indirect_dma=True)
    assert len(in_ap) == 1 and len(out_ap) == 1
    off = eng.lower_ap_dma(offset_ap)
    assert len(off) == 1
    in_ap.append(off[0])

    coef = 1
    for s in out.shape[1:]:
        coef *= s
    out_ap[0].dynamic_ap_info = mb.DynamicAccessPatternInfo(
        c=0,
        actual_ap=in_.ap,
        indirect_dim_max_index=out.shape[0],
        offset_expr=[
            mb.DynamicAccessPatternOffsetExpr(
                coef=coef,
                aff_expr=mb.DynamicAccessPatternOffsetExprAffExpr(
                    kind="IndirectArgId", arg_id=1
                ),
            )
        ],
    )
    bc = [eng.lower_val_access(eng.to_reg(bounds))]
    qname = f"qPoolDynamic{queue_num or ''}"
    return eng.add_instruction(
        mb.InstDMACopy(
            name=eng.bass.get_next_instruction_name(),
            queue=qname,
            mode="Copy",
            ins=in_ap + bc,
            outs=out_ap,
            oob_is_err=False,
            cce_op=mb.AluOpType.bypass,
        )
    )


def _build_program():
    nc = bacc.Bacc(
        "TRN2", target_bir_lowering=False, debug=False, num_swdge_queues=NSWQ
    )

    x_d = nc.dram_tensor("x", [NS * C, HW], F32, kind="ExternalInput").ap()
    w1_d = nc.dram_tensor("w1", [C, HID], F32, kind="ExternalInput").ap()
    b1_d = nc.dram_tensor("b1", [HID, 1], F32, kind="ExternalInput").ap()
    # w2aug = [W2; 2*b2] so layer 2 + both bias adds fold into one K=33 matmul
    w2_d = nc.dram_tensor("w2aug", [HID + 1, C * D], F32, kind="ExternalInput").ap()
    ident_d = nc.dram_tensor("ident", [P, P], F32, kind="ExternalInput").ap()
    iota_d = nc.dram_tensor("iota512", [P, C], F32, kind="ExternalInput").ap()
    rhs2_d = nc.dram_tensor("rhs2", [P, 2], F32, kind="ExternalInput").ap()
    out_d = [
        [
            nc.dram_tensor(f"out_{n}_{d}", [BLOCK, HW], F32, kind="ExternalOutput").ap()
            for d in range(D)
        ]
        for n in range(NS)
    ]

    with tile.TileContext(nc) as tc:
        with (
            tc.tile_pool(name="xin", bufs=XBUFS) as xpool,
            tc.tile_pool(name="xstream", bufs=2) as spool,
            tc.tile_pool(name="small", bufs=1) as sm,
            tc.tile_pool(name="mbuf", bufs=2) as mpool,
            tc.tile_pool(name="psum", bufs=1, space="PSUM") as psum,
            tc.tile_pool(name="psumr", bufs=2, space="PSUM") as psumr,
        ):
            # ---- constants / weights into SBUF on the scalar(ACT) ring ----
            w1_sb = sm.tile([P, CT, HID], F32)
            nc.scalar.dma_start(
                out=w1_sb[:], in_=w1_d.rearrange("(c p) m -> p c m", p=P)
            )
            w2_sb = sm.tile([HID + 1, C * D], F32)
            nc.scalar.dma_start(out=w2_sb[:], in_=w2_d)
            b1_sb = sm.tile([HID, 1], F32)
            nc.scalar.dma_start(out=b1_sb[:], in_=b1_d)
            ident_sb = sm.tile([P, P], F32)
            nc.scalar.dma_start(out=ident_sb[:], in_=ident_d)
            iota_sb = sm.tile([P, C], F32)
            nc.scalar.dma_start(out=iota_sb[:], in_=iota_d)
            rhs2_sb = sm.tile([P, 2], F32)
            nc.scalar.dma_start(out=rhs2_sb[:], in_=rhs2_d)

            # [P, ct, {avg partials AVGCH*li+c, max cols nS*AVGCH+li}]
            nsz = [len(g) for g in GROUPS]
            pools = [
                sm.tile([P, CT, nsz[g] * (AVGCH + 1)], F32, name=f"pools{g}")
                for g in range(2)
            ]
            scratch = sm.tile([P, HW // AVGCH], F32)

            # one vals double-buffer shared by both groups (group 1 rewrites
            # the live rows; leftover NEG_FILLs elsewhere are unused)
            vals = [sm.tile([P, C], F32, name=f"vals_{i}") for i in range(2)]
            nc.vector.memset(vals[0][:], 0.0)

            offs_u = [
                sm.tile([P, nsz[g] * D, CT], U32, name=f"offs{g}") for g in range(2)
            ]

            xtiles = {}

            def load_tile(n, ct, eng, stream=False):
                row0 = (n * CT + ct) * P
                pool = spool if stream else xpool
                xt = pool.tile([P, HW], F32, tag="xs" if stream else "xt")
                xtiles[(n, ct)] = xt
                eng.dma_start(out=xt[:], in_=x_d[row0 : row0 + P, :])

            def avg_tile(n, ct, g, li):
                csz = HW // AVGCH
                xt = xtiles[(n, ct)]
                for c in range(AVGCH):
                    col = AVGCH * li + c
                    nc.scalar.activation(
                        out=scratch[:],
                        in_=xt[:, c * csz : (c + 1) * csz],
                        func=mybir.ActivationFunctionType.Copy,
                        scale=1.0 / HW,
                        accum_out=pools[g][:, ct, col : col + 1],
                    )

            def max_tile(n, ct, g, li):
                mc = nsz[g] * AVGCH + li
                nc.vector.reduce_max(
                    out=pools[g][:, ct, mc : mc + 1],
                    in_=xtiles[(n, ct)][:],
                    axis=mybir.AxisListType.X,
                )

            def mlp_group(g):
                """py rows 32d+li = y[sample li of group g]."""
                nS = nsz[g]
                ncols = nS * (AVGCH + 1)
                ph = psum.tile([HID, ncols], F32, space="PSUM", tag="ph")
                for ct in range(CT):
                    nc.tensor.matmul(
                        out=ph[:],
                        lhsT=w1_sb[:, ct, :],
                        rhs=pools[g][:, ct, :],
                        start=(ct == 0),
                        stop=(ct == CT - 1),
                    )
                # W1.T is linear: sum the avg partial columns after the matmul
                avgs = sm.tile([HID, nS], F32, name=f"avgs{g}")
                for li in range(nS):
                    nc.vector.reduce_sum(
                        out=avgs[:, li : li + 1],
                        in_=ph[:, AVGCH * li : AVGCH * (li + 1)],
                        axis=mybir.AxisListType.X,
                    )
                hTa = sm.tile([HID, nS], F32, name=f"hTa{g}")
                hTm = sm.tile([HID, nS], F32, name=f"hTm{g}")
                nc.scalar.activation(
                    out=hTa[:], in_=avgs[:],
                    func=mybir.ActivationFunctionType.Relu, bias=b1_sb[:, :],
                )
                nc.scalar.activation(
                    out=hTm[:], in_=ph[:, nS * AVGCH : nS * AVGCH + nS],
                    func=mybir.ActivationFunctionType.Relu, bias=b1_sb[:, :],
                )
                hsum = sm.tile([HID, nS], F32, name=f"hsum{g}")
                nc.vector.tensor_add(out=hsum[:], in0=hTa[:], in1=hTm[:])
                hw_t = sm.tile([HID + 1, P], F32, name=f"hw{g}")
                nc.vector.memset(hw_t[:], 0.0)
                nc.vector.memset(hw_t[32:33, :], 1.0)
                for d in range(D):
                    nc.vector.tensor_copy(
                        out=hw_t[0:HID, 32 * d : 32 * d + nS], in_=hsum[:]
                    )

                py = psum.tile([P, C * D], F32, space="PSUM", tag="py")
                for s in range(C * D // 512):
                    sl = slice(s * 512, (s + 1) * 512)
                    nc.tensor.matmul(
                        out=py[:, sl], lhsT=hw_t[:], rhs=w2_sb[:, sl],
                        start=True, stop=True,
                    )
                va = vals[0]
                for d in range(D):
                    nc.vector.tensor_copy(
                        out=va[32 * d : 32 * d + nS, :],
                        in_=py[32 * d : 32 * d + nS, d :: D],
                    )

            def topk_group(g):
                """ptf column 32d+li = topk channel ids (rank k on partitions)."""
                topk_idx = sm.tile([P, BLOCK], U32, name=f"tki{g}")
                maxv = sm.tile([P, 8], F32, name=f"maxv{g}")
                cur, nxt = vals
                for k in range(BLOCK // 8):
                    nc.vector.max(out=maxv[:], in_=cur[:])
                    nc.vector.max_index(
                        out=topk_idx[:, 8 * k : 8 * k + 8],
                        in_max=maxv[:],
                        in_values=cur[:],
                    )
                    if k < BLOCK // 8 - 1:
                        nc.vector.match_replace(
                            out=nxt[:], in_to_replace=maxv[:], in_values=cur[:],
                            imm_value=NEG_FILL,
                        )
                        cur, nxt = nxt, cur

                idx_f = sm.tile([P, BLOCK], F32, name=f"idxf{g}")
                nc.vector.tensor_copy(out=idx_f[:], in_=topk_idx[:])
                pt = psum.tile([P, P], F32, space="PSUM", tag="pt")
                nc.tensor.transpose(out=pt[:], in_=idx_f[:], identity=ident_sb[:])
                ptf = sm.tile([P, P], F32, name=f"ptf{g}")
                nc.vector.tensor_copy(out=ptf[:], in_=pt[:])
                return ptf

            def inverse_group(g, ptf):
                """offs_u[g][:, li*D+d, ct] = output row per channel."""
                for li in range(nsz[g]):
                    for d in range(D):
                        col = 32 * d + li
                        j = li * D + d
                        m = mpool.tile([P, C], F32, tag="m")
                        nc.vector.tensor_scalar(
                            out=m[:], in0=iota_sb[:],
                            scalar1=ptf[:, col : col + 1], scalar2=None,
                            op0=mybir.AluOpType.is_equal,
                        )
                        psR = psumr.tile([P, CT, 2], F32, space="PSUM", tag="psr")
                        for ct in range(CT):
                            nc.tensor.matmul(
                                out=psR[:, ct, :],
                                lhsT=m[:, ct * P : (ct + 1) * P],
                                rhs=rhs2_sb[:],
                                start=True, stop=True,
                            )
                        # offs = rank*sel + BIG*(1-sel)
                        tmp = sm.tile([P, CT], F32, name=f"tmp{g}")
                        nc.vector.tensor_scalar(
                            out=tmp[:], in0=psR[:, :, 1],
                            scalar1=-BIG, scalar2=BIG,
                            op0=mybir.AluOpType.mult,
                            op1=mybir.AluOpType.add,
                        )
                        offs_f = sm.tile([P, CT], F32, name=f"offsf{g}")
                        nc.vector.tensor_add(
                            out=offs_f[:], in0=tmp[:], in1=psR[:, :, 0]
                        )
                        nc.vector.tensor_copy(
                            out=offs_u[g][:, j, :], in_=offs_f[:]
                        )

            sc_counter = [0]

            def scatter_one(g, li, n, d, ct):
                j = li * D + d
                qn = sc_counter[0] % NSWQ
                sc_counter[0] += 1
                _indirect_scatter_q(
                    nc.gpsimd,
                    out=out_d[n][d][:, :],
                    offset_ap=offs_u[g][:, j, ct : ct + 1],
                    in_=xtiles[(n, ct)][:],
                    bounds=BLOCK - 1,
                    queue_num=qn,
                )

            # ================= emission (== engine program order) ===========
            # group A loads: s0, s1, s2ct0 retained (even ct sync, odd ACT);
            # s2ct1/ct2 stream through the 2 stream buffers, ct3 below (its
            # load must follow ct1's pooling to reuse that buffer)
            # streamed s2ct1/ct2 first on their rings: their pooling gates
            # the stream-buffer reuse chain (s2ct3, then s3ct0/ct1)
            load_tile(2, 1, nc.scalar, stream=True)
            load_tile(2, 2, nc.sync, stream=True)
            for n in (0, 1):
                for ct in range(CT):
                    load_tile(n, ct, nc.scalar if ct % 2 else nc.sync)
            load_tile(2, 0, nc.sync)

            # streamed tiles' pooling first: it gates stream-buffer reuse
            for ct in (1, 2):
                avg_tile(2, ct, 0, 2)
            for li, n in enumerate((0, 1)):
                for ct in range(CT):
                    avg_tile(n, ct, 0, li)
            avg_tile(2, 0, 0, 2)
            for ct in (1, 2):
                max_tile(2, ct, 0, 2)
            for li, n in enumerate((0, 1)):
                for ct in range(CT):
                    max_tile(n, ct, 0, li)
            max_tile(2, 0, 0, 2)

            load_tile(2, 3, nc.sync, stream=True)   # reuses ct1's buffer
            avg_tile(2, 3, 0, 2)
            max_tile(2, 3, 0, 2)

            # s3ct0/ct1 stream in during group A's topk (buffers free once
            # s2ct3 is pooled); they stay resident for scatter B
            load_tile(3, 0, nc.sync, stream=True)
            load_tile(3, 1, nc.sync, stream=True)

            mlp_group(0)
            ptf0 = topk_group(0)
            inverse_group(0, ptf0)

            # gated loads: s3's remainder first (pooling is tail-critical),
            # then the s2ct1-3 reloads (only needed late in scatter A)
            load_tile(3, 2, nc.sync)
            load_tile(3, 3, nc.scalar)
            for ct in (1, 2, 3):
                load_tile(2, ct, nc.sync)           # reload into retained pool
            for ct in range(CT):
                avg_tile(3, ct, 1, 0)

            # scatter A: s0/s1 ct-major round-robin, then s2 (reloads last)
            for ct in range(CT):
                for li in (0, 1):
                    for d in range(D):
                        scatter_one(0, li, li, d, ct)
            for ct in range(CT):
                for d in range(D):
                    scatter_one(0, 2, 2, d, ct)

            for ct in range(CT):
                max_tile(3, ct, 1, 0)
            mlp_group(1)
            ptf1 = topk_group(1)
            inverse_group(1, ptf1)
            for ct in range(CT):
                for d in range(D):
                    scatter_one(1, 0, 3, d, ct)

    nc.compile()
    return nc


_NC_CACHE = None


def _get_nc():
    global _NC_CACHE
    if _NC_CACHE is None:
        _NC_CACHE = _build_program()
    return _NC_CACHE


def _make_in_maps(x, W1, b1, W2, b2):
    x = np.ascontiguousarray(np.asarray(x, dtype=np.float32)).reshape(N_FULL, C, HW)
    W1 = np.asarray(W1, dtype=np.float32)
    b1 = np.asarray(b1, dtype=np.float32).reshape(HID, 1)
    W2 = np.asarray(W2, dtype=np.float32)
    b2 = np.asarray(b2, dtype=np.float32).reshape(1, C * D)
    w2aug = np.ascontiguousarray(np.vstack([W2, 2.0 * b2]))
    ident = np.eye(P, dtype=np.float32)
    iota512 = np.tile(np.arange(C, dtype=np.float32), (P, 1))
    rhs2 = np.stack(
        [np.arange(P, dtype=np.float32), np.ones(P, dtype=np.float32)], axis=1
    )
    rhs2 = np.ascontiguousarray(rhs2)
    in_maps = []
    for core in range(N_CORES):
        shard = x[core * NS : (core + 1) * NS].reshape(NS * C, HW)
        in_maps.append(
            {
                "x": np.ascontiguousarray(shard),
                "w1": W1,
                "b1": b1,
                "w2aug": w2aug,
                "ident": ident,
                "iota512": iota512,
                "rhs2": rhs2,
            }
        )
    return in_maps


def run(inputs, trace=False, **kwargs):
    """Run the SPMD kernel; returns (full_output, BassKernelResults)."""
    nc = _get_nc()
    in_maps = _make_in_maps(
        inputs["x"], inputs["W1"], inputs["b1"], inputs["W2"], inputs["b2"]
    )
    res = run_bass_kernel_spmd(
        nc, in_maps, core_ids=list(range(N_CORES)), trace=trace, **kwargs
    )
    out = np.empty((N_FULL, C, 64, 64), dtype=np.float32)
    for core in range(N_CORES):
        r = res.results[core]
        for n in range(NS):
            for d in range(D):
                blk = r[f"out_{n}_{d}"].reshape(BLOCK, 64, 64)
                out[core * NS + n, d * BLOCK : (d + 1) * BLOCK] = blk
    return out, res


def kernel(**inputs) -> np.ndarray:
    out, _ = run(inputs)
    return out


# revision 39
# speedup vs baseline: 1.1525x; 1.0097x over previous
"""Trainium2 Bass kernel for nn_CutoffModule (CBAM-style channel gate + topk gather).

Reference computation (per sample):
    avg/max spatial pooling -> shared 2-layer MLP -> sum -> sigmoid -> attn [C, D]
    per scale d: top-128 channels (sorted desc) -> gather those channels of x.

Sharding: data-parallel over N across 8 cores (4 samples/core); MLP weights
replicated. Entirely self-contained: hardcodes N=32, C=512, H=W=64, D=4, r=16.

Strategy: x is read from HBM once and kept in SBUF; the kernel computes the
INVERSE permutation (channel -> output rank, OOB sentinel when unselected) and
scatters x tiles straight to per-(sample, scale) output tensors with
indirect_dma_start (out_offset + bounds_check skip).  64 MiB HBM traffic/core
instead of the gather baseline's 96 MiB.

Samples are processed in asymmetric groups {0,1,2} then {3}: the big group's
24 MiB scatter keeps HBM busy through the small group's topk window.  SBUF
holds 10 retained x tiles + 1 streaming tile; sample 2's last two tiles are
pooled from the stream and re-loaded later for their scatter.

Notes:
- sigmoid is strictly monotonic, so top_k(sigmoid(y)) == top_k(y).
- topk row (d, li) lives on SBUF partition 32*d + li (engine writes must
  start at partition multiples of 32).
- inverse permutation per (sample, scale): one-hot is_equal + rank/selected
  matmul against [iota128, ones]; unselected channels get row id BIG=200 and
  are skipped by the DMA bounds check.
- every (sample, scale) block is a separate DRAM tensor: scatters to a shared
  tensor are WAW-chained by the tile framework and serialize.
- gpsimd (Pool) issues all scatters; it is a slow DSP for compute and cannot
  touch PSUM, so all element-wise work stays on DVE/ACT.
"""

import numpy as np

import concourse.bacc as bacc
import concourse.bass as bass
import concourse.tile as tile
from concourse import mybir
from concourse.bass_utils import run_bass_kernel_spmd

# Problem constants (hardcoded per harness contract)
N_FULL = 32
C = 512
HW = 64 * 64          # 4096
D = 4                 # depth scales
BLOCK = C // D        # 128
HID = C // 16         # 32  (MLP hidden)
N_CORES = 8
NS = N_FULL // N_CORES  # 4 samples per core
P = 128               # SBUF partitions
CT = C // P           # 4 channel tiles per sample
NEG_FILL = -1e30
BIG = 200.0           # OOB sentinel offset (> BLOCK-1)
XBUFS = 9             # retained x tile buffers (16 KiB/partition each)
AVGCH = 4             # avg-pool sub-chunks per tile (shrinks the ACT scratch)
NSWQ = 2              # SWDGE queues for scatters (round-robin)

GROUPS = [(0, 1, 2), (3,)]

F32 = mybir.dt.float32
U32 = mybir.dt.uint32


def _indirect_scatter_q(eng, out, offset_ap, in_, bounds, queue_num):
    """indirect_dma_start (out_offset form) with a selectable SWDGE queue.

    Replica of bass.BassEngine.indirect_dma_start's scatter path; the library
    hardcodes queue 0 ("qPoolDynamic"), which serializes all scatters behind
    one 8-deep completion-semaphore ring.
    """
    mb = mybir
    assert isinstance(out.offset, int) and out.offset == 0
    out_ap = eng.lower_ap_dma(out, for_indirect_dma=True)
    in_ap = eng.lower_ap_dma(in_, for_indirect_dma=True)
    assert len(in_ap) == 1 and len(out_ap) == 1
    off = eng.lower_ap_dma(offset_ap)
    assert len(off) == 1
    in_ap.append(off[0])

    coef = 1
    for s in out.shape[1:]:
        coef *= s
    out_ap[0].dynamic_ap_info = mb.DynamicAccessPatternInfo(
        c=0,
        actual_ap=in_.ap,
        indirect_dim_max_index=out.shape[0],
        offset_expr=[
            mb.DynamicAccessPatternOffsetExpr(
                coef=coef,
                aff_expr=mb.DynamicAccessPatternOffsetExprAffExpr(
                    kind="IndirectArgId", arg_id=1
                ),
            )
        ],
    )
    bc = [eng.lower_val_access(eng.to_reg(bounds))]
    qname = f"qPoolDynamic{queue_num or ''}"
    return eng.add_instruction(
        mb.InstDMACopy(
            name=eng.bass.get_next_instruction_name(),
            queue=qname,
            mode="Copy",
            ins=in_ap + bc,
            outs=out_ap,
            oob_is_err=False,
            cce_op=mb.AluOpType.bypass,
            single_packet=True,
        )
    )


def _build_program():
    nc = bacc.Bacc(
        "TRN2", target_bir_lowering=False, debug=False, num_swdge_queues=NSWQ
    )

    x_d = nc.dram_tensor("x", [NS * C, HW], F32, kind="ExternalInput").ap()
    w1_d = nc.dram_tensor("w1", [C, HID], F32, kind="ExternalInput").ap()
    b1_d = nc.dram_tensor("b1", [HID, 1], F32, kind="ExternalInput").ap()
    # w2aug = [W2; 2*b2] so layer 2 + both bias adds fold into one K=33 matmul
    w2_d = nc.dram_tensor("w2aug", [HID + 1, C * D], F32, kind="ExternalInput").ap()
    ident_d = nc.dram_tensor("ident", [P, P], F32, kind="ExternalInput").ap()
    iota_d = nc.dram_tensor("iota512", [P, C], F32, kind="ExternalInput").ap()
    rhs2_d = nc.dram_tensor("rhs2", [P, 2], F32, kind="ExternalInput").ap()
    out_d = [
        [
            nc.dram_tensor(f"out_{n}_{d}", [BLOCK, HW], F32, kind="ExternalOutput").ap()
            for d in range(D)
        ]
        for n in range(NS)
    ]

    with tile.TileContext(nc) as tc:
        with (
            tc.tile_pool(name="xin", bufs=XBUFS) as xpool,
            tc.tile_pool(name="xstream", bufs=2) as spool,
            tc.tile_pool(name="small", bufs=1) as sm,
            tc.tile_pool(name="mbuf", bufs=2) as mpool,
            tc.tile_pool(name="psum", bufs=1, space="PSUM") as psum,
            tc.tile_pool(name="psumr", bufs=2, space="PSUM") as psumr,
        ):
            # ---- constants / weights into SBUF on the scalar(ACT) ring ----
            w1_sb = sm.tile([P, CT, HID], F32)
            nc.scalar.dma_start(
                out=w1_sb[:], in_=w1_d.rearrange("(c p) m -> p c m", p=P)
            )
            w2_sb = sm.tile([HID + 1, C * D], F32)
            nc.scalar.dma_start(out=w2_sb[:], in_=w2_d)
            b1_sb = sm.tile([HID, 1], F32)
            nc.scalar.dma_start(out=b1_sb[:], in_=b1_d)
            ident_sb = sm.tile([P, P], F32)
            nc.scalar.dma_start(out=ident_sb[:], in_=ident_d)
            iota_sb = sm.tile([P, C], F32)
            nc.scalar.dma_start(out=iota_sb[:], in_=iota_d)
            rhs2_sb = sm.tile([P, 2], F32)
            nc.scalar.dma_start(out=rhs2_sb[:], in_=rhs2_d)

            # [P, ct, {avg partials AVGCH*li+c, max cols nS*AVGCH+li}]
            nsz = [len(g) for g in GROUPS]
            pools = [
                sm.tile([P, CT, nsz[g] * (AVGCH + 1)], F32, name=f"pools{g}")
                for g in range(2)
            ]
            scratch = sm.tile([P, HW // AVGCH], F32)

            # one vals double-buffer shared by both groups (group 1 rewrites
            # the live rows; leftover NEG_FILLs elsewhere are unused)
            vals = [sm.tile([P, C], F32, name=f"vals_{i}") for i in range(2)]
            nc.vector.memset(vals[0][:], 0.0)

            offs_u = [
                sm.tile([P, nsz[g] * D, CT], U32, name=f"offs{g}") for g in range(2)
            ]

            xtiles = {}

            def load_tile(n, ct, eng, stream=False):
                row0 = (n * CT + ct) * P
                pool = spool if stream else xpool
                xt = pool.tile([P, HW], F32, tag="xs" if stream else "xt")
                xtiles[(n, ct)] = xt
                eng.dma_start(out=xt[:], in_=x_d[row0 : row0 + P, :])

            def avg_tile(n, ct, g, li):
                csz = HW // AVGCH
                xt = xtiles[(n, ct)]
                for c in range(AVGCH):
                    col = AVGCH * li + c
                    nc.scalar.activation(
                        out=scratch[:],
                        in_=xt[:, c * csz : (c + 1) * csz],
                        func=mybir.ActivationFunctionType.Copy,
                        scale=1.0 / HW,
                        accum_out=pools[g][:, ct, col : col + 1],
                    )

            def max_tile(n, ct, g, li):
                mc = nsz[g] * AVGCH + li
                nc.vector.reduce_max(
                    out=pools[g][:, ct, mc : mc + 1],
                    in_=xtiles[(n, ct)][:],
                    axis=mybir.AxisListType.X,
                )

            def mlp_group(g):
                """py rows 32d+li = y[sample li of group g]."""
                nS = nsz[g]
                ncols = nS * (AVGCH + 1)
                ph = psum.tile([HID, ncols], F32, space="PSUM", tag="ph")
                for ct in range(CT):
                    nc.tensor.matmul(
                        out=ph[:],
                        lhsT=w1_sb[:, ct, :],
                        rhs=pools[g][:, ct, :],
                        start=(ct == 0),
                        stop=(ct == CT - 1),
                    )
                # W1.T is linear: sum the avg partial columns after the matmul
                avgs = sm.tile([HID, nS], F32, name=f"avgs{g}")
                for li in range(nS):
                    nc.vector.reduce_sum(
                        out=avgs[:, li : li + 1],
                        in_=ph[:, AVGCH * li : AVGCH * (li + 1)],
                        axis=mybir.AxisListType.X,
                    )
                hTa = sm.tile([HID, nS], F32, name=f"hTa{g}")
                hTm = sm.tile([HID, nS], F32, name=f"hTm{g}")
                nc.scalar.activation(
                    out=hTa[:], in_=avgs[:],
                    func=mybir.ActivationFunctionType.Relu, bias=b1_sb[:, :],
                )
                nc.scalar.activation(
                    out=hTm[:], in_=ph[:, nS * AVGCH : nS * AVGCH + nS],
                    func=mybir.ActivationFunctionType.Relu, bias=b1_sb[:, :],
                )
                hsum = sm.tile([HID, nS], F32, name=f"hsum{g}")
                nc.vector.tensor_add(out=hsum[:], in0=hTa[:], in1=hTm[:])
                hw_t = sm.tile([HID + 1, P], F32, name=f"hw{g}")
                nc.vector.memset(hw_t[:], 0.0)
                nc.vector.memset(hw_t[32:33, :], 1.0)
                for d in range(D):
                    nc.vector.tensor_copy(
                        out=hw_t[0:HID, 32 * d : 32 * d + nS], in_=hsum[:]
                    )

                py = psum.tile([P, C * D], F32, space="PSUM", tag="py")
                for s in range(C * D // 512):
                    sl = slice(s * 512, (s + 1) * 512)
                    nc.tensor.matmul(
                        out=py[:, sl], lhsT=hw_t[:], rhs=w2_sb[:, sl],
                        start=True, stop=True,
                    )
                va = vals[0]
                for d in range(D):
                    nc.vector.tensor_copy(
                        out=va[32 * d : 32 * d + nS, :],
                        in_=py[32 * d : 32 * d + nS, d :: D],
                    )

            def topk_group(g):
                """ptf column 32d+li = topk channel ids (rank k on partitions)."""
                topk_idx = sm.tile([P, BLOCK], U32, name=f"tki{g}")
                maxv = sm.tile([P, 8], F32, name=f"maxv{g}")
                cur, nxt = vals
                for k in range(BLOCK // 8):
                    nc.vector.max(out=maxv[:], in_=cur[:])
                    nc.vector.max_index(
                        out=topk_idx[:, 8 * k : 8 * k + 8],
                        in_max=maxv[:],
                        in_values=cur[:],
                    )
                    if k < BLOCK // 8 - 1:
                        nc.vector.match_replace(
                            out=nxt[:], in_to_replace=maxv[:], in_values=cur[:],
                            imm_value=NEG_FILL,
                        )
                        cur, nxt = nxt, cur

                idx_f = sm.tile([P, BLOCK], F32, name=f"idxf{g}")
                nc.vector.tensor_copy(out=idx_f[:], in_=topk_idx[:])
                pt = psum.tile([P, P], F32, space="PSUM", tag="pt")
                nc.tensor.transpose(out=pt[:], in_=idx_f[:], identity=ident_sb[:])
                ptf = sm.tile([P, P], F32, name=f"ptf{g}")
                nc.vector.tensor_copy(out=ptf[:], in_=pt[:])
                return ptf

            def inverse_group(g, ptf):
                """offs_u[g][:, li*D+d, ct] = output row per channel."""
                for li in range(nsz[g]):
                    for d in range(D):
                        col = 32 * d + li
                        j = li * D + d
                        m = mpool.tile([P, C], F32, tag="m")
                        nc.vector.tensor_scalar(
                            out=m[:], in0=iota_sb[:],
                            scalar1=ptf[:, col : col + 1], scalar2=None,
                            op0=mybir.AluOpType.is_equal,
                        )
                        psR = psumr.tile([P, CT, 2], F32, space="PSUM", tag="psr")
                        for ct in range(CT):
                            nc.tensor.matmul(
                                out=psR[:, ct, :],
                                lhsT=m[:, ct * P : (ct + 1) * P],
                                rhs=rhs2_sb[:],
                                start=True, stop=True,
                            )
                        # offs = rank*sel + BIG*(1-sel)
                        tmp = sm.tile([P, CT], F32, name=f"tmp{g}")
                        nc.vector.tensor_scalar(
                            out=tmp[:], in0=psR[:, :, 1],
                            scalar1=-BIG, scalar2=BIG,
                            op0=mybir.AluOpType.mult,
                            op1=mybir.AluOpType.add,
                        )
                        offs_f = sm.tile([P, CT], F32, name=f"offsf{g}")
                        nc.vector.tensor_add(
                            out=offs_f[:], in0=tmp[:], in1=psR[:, :, 0]
                        )
                        nc.vector.tensor_copy(
                            out=offs_u[g][:, j, :], in_=offs_f[:]
                        )

            sc_counter = [0]

            def scatter_one(g, li, n, d, ct):
                j = li * D + d
                qn = sc_counter[0] % NSWQ
                sc_counter[0] += 1
                _indirect_scatter_q(
                    nc.gpsimd,
                    out=out_d[n][d][:, :],
                    offset_ap=offs_u[g][:, j, ct : ct + 1],
                    in_=xtiles[(n, ct)][:],
                    bounds=BLOCK - 1,
                    queue_num=qn,
                )

            # ================= emission (== engine program order) ===========
            # group A loads: s0, s1, s2ct0 retained (even ct sync, odd ACT);
            # s2ct1/ct2 stream through the 2 stream buffers, ct3 below (its
            # load must follow ct1's pooling to reuse that buffer)
            # streamed s2ct1/ct2 first on their rings: their pooling gates
            # the stream-buffer reuse chain (s2ct3, then s3ct0/ct1)
            load_tile(2, 1, nc.scalar, stream=True)
            load_tile(2, 2, nc.sync, stream=True)
            for n in (0, 1):
                for ct in range(CT):
                    load_tile(n, ct, nc.scalar if ct % 2 else nc.sync)
            load_tile(2, 0, nc.sync)

            # streamed tiles' pooling first: it gates stream-buffer reuse
            for ct in (1, 2):
                avg_tile(2, ct, 0, 2)
            for li, n in enumerate((0, 1)):
                for ct in range(CT):
                    avg_tile(n, ct, 0, li)
            avg_tile(2, 0, 0, 2)
            for ct in (1, 2):
                max_tile(2, ct, 0, 2)
            for li, n in enumerate((0, 1)):
                for ct in range(CT):
                    max_tile(n, ct, 0, li)
            max_tile(2, 0, 0, 2)

            load_tile(2, 3, nc.sync, stream=True)   # reuses ct1's buffer
            avg_tile(2, 3, 0, 2)
            max_tile(2, 3, 0, 2)

            # s3ct0/ct1 stream in during group A's topk (buffers free once
            # s2ct3 is pooled); they stay resident for scatter B
            load_tile(3, 0, nc.sync, stream=True)
            load_tile(3, 1, nc.sync, stream=True)

            mlp_group(0)
            ptf0 = topk_group(0)
            inverse_group(0, ptf0)

            # gated loads: s3's remainder first (pooling is tail-critical),
            # then the s2ct1-3 reloads (only needed late in scatter A)
            load_tile(3, 2, nc.sync)
            load_tile(3, 3, nc.scalar)
            for ct in (1, 2, 3):
                load_tile(2, ct, nc.sync)           # reload into retained pool
            for ct in range(CT):
                avg_tile(3, ct, 1, 0)

            # scatter A: s0/s1 ct-major round-robin, then s2 (reloads last)
            for ct in range(CT):
                for li in (0, 1):
                    for d in range(D):
                        scatter_one(0, li, li, d, ct)
            for ct in range(CT):
                for d in range(D):
                    scatter_one(0, 2, 2, d, ct)

            for ct in range(CT):
                max_tile(3, ct, 1, 0)
            mlp_group(1)
            ptf1 = topk_group(1)
            inverse_group(1, ptf1)
            for ct in range(CT):
                for d in range(D):
                    scatter_one(1, 0, 3, d, ct)

    nc.compile()
    return nc


_NC_CACHE = None


def _get_nc():
    global _NC_CACHE
    if _NC_CACHE is None:
        _NC_CACHE = _build_program()
    return _NC_CACHE


def _make_in_maps(x, W1, b1, W2, b2):
    x = np.ascontiguousarray(np.asarray(x, dtype=np.float32)).reshape(N_FULL, C, HW)
    W1 = np.asarray(W1, dtype=np.float32)
    b1 = np.asarray(b1, dtype=np.float32).reshape(HID, 1)
    W2 = np.asarray(W2, dtype=np.float32)
    b2 = np.asarray(b2, dtype=np.float32).reshape(1, C * D)
    w2aug = np.ascontiguousarray(np.vstack([W2, 2.0 * b2]))
    ident = np.eye(P, dtype=np.float32)
    iota512 = np.tile(np.arange(C, dtype=np.float32), (P, 1))
    rhs2 = np.stack(
        [np.arange(P, dtype=np.float32), np.ones(P, dtype=np.float32)], axis=1
    )
    rhs2 = np.ascontiguousarray(rhs2)
    in_maps = []
    for core in range(N_CORES):
        shard = x[core * NS : (core + 1) * NS].reshape(NS * C, HW)
        in_maps.append(
            {
                "x": np.ascontiguousarray(shard),
                "w1": W1,
                "b1": b1,
                "w2aug": w2aug,
                "ident": ident,
                "iota512": iota512,
                "rhs2": rhs2,
            }
        )
    return in_maps


def run(inputs, trace=False, **kwargs):
    """Run the SPMD kernel; returns (full_output, BassKernelResults)."""
    nc = _get_nc()
    in_maps = _make_in_maps(
        inputs["x"], inputs["W1"], inputs["b1"], inputs["W2"], inputs["b2"]
    )
    res = run_bass_kernel_spmd(
        nc, in_maps, core_ids=list(range(N_CORES)), trace=trace, **kwargs
    )
    out = np.empty((N_FULL, C, 64, 64), dtype=np.float32)
    for core in range(N_CORES):
        r = res.results[core]
        for n in range(NS):
            for d in range(D):
                blk = r[f"out_{n}_{d}"].reshape(BLOCK, 64, 64)
                out[core * NS + n, d * BLOCK : (d + 1) * BLOCK] = blk
    return out, res


def kernel(**inputs) -> np.ndarray:
    out, _ = run(inputs)
    return out
